# revision 3
# baseline (speedup 1.0000x reference)
"""GCNII conv (gnn_message_passing) Trainium2 Bass kernel.

Strategy (8-way node sharding, DRAM-resident relabeled feature table):
  - Host: relabel node-rows per core so gather indices fit int16 windows:
    "lo" window = table rows [0, 32768) (row 0 zeros), "hi" window = rows
    [32768, 40960) (row 32768 zeros).  Rows are assigned so each node has at
    most s_hi refs into the hi window.  Nodes with a hi ref are permuted to
    the FRONT of the core's shard so only the leading tiles need a hi gather
    plane; tiles made purely of hi-nodes drop their all-padding 16th lo
    plane.  The inverse permutation is applied to the output on host.
  - Device: dma_gather in transpose mode sources 256B rows directly from the
    DRAM table (no SBUF table copy); PE sums the neighbor slots plus self via
    bf16 identity matmuls into PSUM (fp32 exact), then the GCNII combine is
    bf16 GEMMs (M1s = (s1*I + beta*W1)/deg on gather_sum + x_self, M2 =
    s2*I + beta*W2 on x_0) plus bias+ReLU on the activation engine.
"""

import numpy as np
import ml_dtypes

import concourse.bacc as bacc
import concourse.mybir as mybir
from concourse.tile import TileContext
from concourse.bass_utils import run_bass_kernel_spmd

BF16 = ml_dtypes.bfloat16
F32 = np.float32

ALPHA = 0.1
BETA = float(np.log(0.5 / 4 + 1.0))
DEG_K = 16           # neighbors per node (w/o self loop)
C = 128              # channels
P = 128              # partitions

N_FULL = 40000
N_CORES = 8
LO_CAP_FULL = 32768  # rows in lo window (incl zero row at local id 0)

GATHER_CHUNK = 896   # idxs per dma_gather instruction (ucode cap ~992, %128)
SINGLE_PACKET = True
TAIL_PREFETCH = False  # issue tail-tile gathers early (helps only if DMA slack)
SPLIT_TAIL = True      # split the last tile to shorten the exposed end chain
SMALL_FIRST = False    # head bubbles block the critical path; keep small tiles last
RAMP_FIRST = False     # progressive tile-0 chunks regressed in sim
SHARED_REGS = True     # one num_idxs register per distinct chunk size


# --------------------------------------------------------------------------
# host-side preparation
# --------------------------------------------------------------------------

def _choose_hi_rows(refs, owners, n_rows, nsh, hi_needed, s_hi):
    """Pick `hi_needed` rows for the hi window s.t. no node has more than
    `s_hi` references into the hi window.  Prefers cold rows."""
    counts = np.bincount(refs, minlength=n_rows)
    order = np.argsort(counts, kind="stable")
    si = np.argsort(refs, kind="stable")
    owners_s = owners[si]
    starts = np.searchsorted(refs[si], np.arange(n_rows))
    ends = np.searchsorted(refs[si], np.arange(n_rows) + 1)

    is_hi = np.zeros(n_rows, dtype=bool)
    node_cnt = np.zeros(nsh, dtype=np.int64)
    zero_rows = order[counts[order] == 0]
    take = zero_rows[: min(len(zero_rows), hi_needed)]
    is_hi[take] = True
    n_hi = len(take)
    if n_hi < hi_needed:
        for r in order:
            if counts[r] == 0 or is_hi[r]:
                continue
            ow = owners_s[starts[r]:ends[r]]
            u, m = np.unique(ow, return_counts=True)
            if (node_cnt[u] + m <= s_hi).all():
                node_cnt[u] += m
                is_hi[r] = True
                n_hi += 1
                if n_hi == hi_needed:
                    break
    if n_hi != hi_needed:
        return None
    return is_hi


def _choose_hi_rows_masked(refs, owners, n_rows, nsh, hi_needed, s_hi, counts):
    """_choose_hi_rows with externally supplied counts (paired rows masked)."""
    order = np.argsort(counts, kind="stable")
    si = np.argsort(refs, kind="stable")
    owners_s = owners[si]
    starts = np.searchsorted(refs[si], np.arange(n_rows))
    ends = np.searchsorted(refs[si], np.arange(n_rows) + 1)

    is_hi = np.zeros(n_rows, dtype=bool)
    node_cnt = np.zeros(nsh, dtype=np.int64)
    zero_rows = order[counts[order] == 0]
    take = zero_rows[: min(len(zero_rows), hi_needed)]
    is_hi[take] = True
    n_hi = len(take)
    if n_hi < hi_needed:
        for r in order:
            if counts[r] == 0 or counts[r] >= (1 << 29) or is_hi[r]:
                continue
            ow = owners_s[starts[r]:ends[r]]
            u, m = np.unique(ow, return_counts=True)
            if (node_cnt[u] + m <= s_hi).all():
                node_cnt[u] += m
                is_hi[r] = True
                n_hi += 1
                if n_hi == hi_needed:
                    break
    if n_hi != hi_needed:
        return None
    return is_hi


def _split_tiles(nsh):
    """Tile sizes; small tail tile shortens the exposed end-of-kernel chain."""
    n_full, left = divmod(nsh, 512)
    if left == 0:
        small = []
    elif SPLIT_TAIL and left > 128 and left % 8 == 0 and (left - 128) % 8 == 0 \
            and left - 128 >= 8:
        small = [128, left - 128] if SMALL_FIRST else [left - 128, 128]
    else:
        small = [left]
    tiles = small + [512] * n_full if SMALL_FIRST else [512] * n_full + small
    assert sum(tiles) == nsh
    return tiles


def _greedy_match(refs_n, n_rows, eligible):
    """Greedy matching on the co-reference graph restricted to eligible rows:
    pair rows that some node references together.  partner[r] = row or -1."""
    nsh, K = refs_n.shape
    us, vs = [], []
    for a in range(K):
        for b in range(a + 1, K):
            u = refs_n[:, a]
            v = refs_n[:, b]
            m = (u != v) & eligible[u] & eligible[v]
            uu = np.minimum(u[m], v[m])
            vv = np.maximum(u[m], v[m])
            us.append(uu)
            vs.append(vv)
    U = np.concatenate(us)
    V = np.concatenate(vs)
    partner = np.full(n_rows, -1, dtype=np.int64)
    rng = np.random.default_rng(0)
    for _ in range(12):
        free = (partner[U] < 0) & (partner[V] < 0)
        U, V = U[free], V[free]
        if len(U) == 0:
            break
        sh = rng.permutation(len(U))
        U, V = U[sh], V[sh]
        o = np.argsort(U, kind="stable")
        U, V = U[o], V[o]
        fu = np.ones(len(U), bool)
        fu[1:] = U[1:] != U[:-1]
        U1, V1 = U[fu], V[fu]
        o2 = np.argsort(V1, kind="stable")
        U1, V1 = U1[o2], V1[o2]
        fv = np.ones(len(V1), bool)
        fv[1:] = V1[1:] != V1[:-1]
        U2, V2 = U1[fv], V1[fv]
        ok = ~np.isin(U2, V2) & ~np.isin(V2, U2)
        partner[U2[ok]] = V2[ok]
        partner[V2[ok]] = U2[ok]
    return partner


def _core_hi_info(x_bf16, idx_shard, nsh, n_rows, lo_cap, s_hi):
    """Phase 1: hi rows, row pairing among lo rows, node permutation."""
    K = idx_shard.shape[1]
    refs_n = idx_shard.astype(np.int64)              # [nsh, K]
    refs = refs_n.reshape(-1)
    owners = np.repeat(np.arange(nsh, dtype=np.int64), K)
    # ids: [zero pair (2)] [paired rows] [unpaired lo rows] | [hi rows]
    hi_needed = max(0, n_rows + 2 - lo_cap)
    if hi_needed > 0:
        is_hi = _choose_hi_rows(refs, owners, n_rows, nsh, hi_needed, s_hi)
        if is_hi is None:
            return None
    else:
        is_hi = np.zeros(n_rows, dtype=bool)

    partner = _greedy_match(refs_n, n_rows, ~is_hi)

    # per-node pair selection over ref occurrences (each occurrence used once)
    used = np.zeros((nsh, K), dtype=bool)
    p_cnt = np.zeros(nsh, dtype=np.int64)
    for a in range(K):
        for b in range(a + 1, K):
            u = refs_n[:, a]
            v = refs_n[:, b]
            cand = (~used[:, a] & ~used[:, b] & (partner[u] == v)
                    & (u != v) & (p_cnt < K // 2))
            if not cand.any():
                continue
            used[cand, a] = True
            used[cand, b] = True
            p_cnt[cand] += 1

    # rows actually pair-fetched: keep ALL matched row pairs in the pair
    # region (a matched row might also be single-fetched; its id stays
    # < 2*(n_pairs+1) < lo_cap so the lo singles window reaches it)
    matched = partner >= 0
    plist = np.where(matched & (partner > np.arange(n_rows)))[0]
    n_pairs = len(plist)
    pair_id = np.full(n_rows, -1, dtype=np.int64)
    pair_id[plist] = 1 + np.arange(n_pairs)          # pair 0 = zero pad pair
    pair_id[partner[plist]] = pair_id[plist]

    lid = np.empty(n_rows, dtype=np.int64)
    lid[plist] = 2 * pair_id[plist]
    lid[partner[plist]] = 2 * pair_id[plist] + 1
    unp_lo = np.where(~matched & ~is_hi)[0]
    base = 2 * (n_pairs + 1)
    lid[unp_lo] = base + np.arange(len(unp_lo))
    hi_rows = np.where(is_hi)[0]
    lid[hi_rows] = lo_cap + 1 + np.arange(len(hi_rows))
    assert base + len(unp_lo) <= lo_cap, (base, len(unp_lo))
    assert n_pairs + 1 <= 16384, n_pairs

    lids_n = lid[refs_n]
    h_n = (lids_n >= lo_cap).sum(axis=1)
    # node order: hi-nodes first, then rest; within each group pair-count desc
    key = (h_n == 0).astype(np.int64) * (K + 2) + (K // 2 - p_cnt)
    perm = np.argsort(key, kind="stable")
    return dict(lid=lid, lids_n=lids_n, h_n=h_n, perm=perm,
                n_hi_nodes=int((h_n > 0).sum()), p_cnt=p_cnt,
                pair_id=pair_id, used=used, refs_n=refs_n,
                n_pairs=n_pairs)


def _pack_core(info, nsh, lo_cap, hi_cap, s_hi, tiles, planes_lo, has_hi,
               planes_pair):
    """Phase 2: build index grids given the shared tile structure."""
    lid = info["lid"]
    perm = info["perm"]
    refs_n = info["refs_n"][perm]        # [nsh, K] global rows, permuted
    used = info["used"][perm]
    h_n = info["h_n"][perm]
    p_cnt = info["p_cnt"][perm]
    pair_id = info["pair_id"]
    K = refs_n.shape[1]
    lids_n = lid[refs_n]

    lo_cols = sum(nt * (pl - 2 * pp) // 16
                  for nt, pl, pp in zip(tiles, planes_lo, planes_pair))
    pr_cols = sum(nt * pp // 16 for nt, pp in zip(tiles, planes_pair))
    n_hi_tiles = int(sum(has_hi))
    lo_idx = np.zeros((16, max(lo_cols, 16)), dtype=np.int16)
    pr_idx = np.zeros((16, max(pr_cols, 16)), dtype=np.int16)
    hi_idx = np.zeros((16, 32 * s_hi * max(n_hi_tiles, 1)), dtype=np.int16)
    off = 0
    lcol = 0
    pcol = 0
    hi_t = 0
    for t, nt in enumerate(tiles):
        pl = planes_lo[t]
        pp = planes_pair[t]
        sl_r = refs_n[off:off + nt]
        sl_u = used[off:off + nt].copy()
        sl_h = h_n[off:off + nt]
        assert (p_cnt[off:off + nt] >= pp).all(), (t, pp)
        # pair slots: first pp pairs of each node -> pair ids
        pvals = np.zeros((nt, pp), dtype=np.int64)
        consumed = np.zeros_like(sl_u)
        if pp:
            taken = np.zeros(nt, dtype=np.int64)
            for a in range(K):
                for b in range(a + 1, K):
                    u = sl_r[:, a]
                    v = sl_r[:, b]
                    cand = (sl_u[:, a] & sl_u[:, b]
                            & ~consumed[:, a] & ~consumed[:, b]
                            & (pair_id[u] >= 0)
                            & (pair_id[u] == pair_id[v]) & (u != v)
                            & (taken < pp))
                    if not cand.any():
                        continue
                    pvals[cand, taken[cand]] = pair_id[u[cand]]
                    consumed[cand, a] = True
                    consumed[cand, b] = True
                    taken[cand] += 1
            assert (taken == pp).all(), (t, pp, taken.min())
            flatp = pvals.T.reshape(-1)
            ncolp = pp * nt // 16
            pr_idx[:, pcol:pcol + ncolp] = flatp.reshape(-1, 16).T
            pcol += ncolp
        # singles: all lo-ref occurrences not consumed as pairs (includes
        # never-paired occurrences and leftover unused pairs)
        n_sing = pl - 2 * pp
        svals = np.zeros((nt, n_sing), dtype=np.int64)
        islo = lids_n[off:off + nt] < lo_cap
        for j_col in range(nt):
            occ = np.where(~consumed[j_col] & islo[j_col])[0]
            vals = lids_n[off + j_col, occ]
            assert len(vals) <= n_sing, (t, j_col, len(vals), n_sing)
            svals[j_col, :len(vals)] = vals
        flat = svals.T.reshape(-1)
        ncol = n_sing * nt // 16
        lo_idx[:, lcol:lcol + ncol] = flat.reshape(-1, 16).T
        lcol += ncol
        # hi plane
        if has_hi[t]:
            assert nt == 512
            hvals = np.zeros((s_hi, nt), dtype=np.int64)
            ishi_n = lids_n[off:off + nt] >= lo_cap
            for j_col in range(nt):
                occ = np.where(ishi_n[j_col])[0]
                assert len(occ) <= s_hi
                for s, o in enumerate(occ):
                    hvals[s, j_col] = lids_n[off + j_col, o] - lo_cap
            hflat = np.zeros(s_hi * 512, dtype=np.int64)
            for s in range(s_hi):
                hflat[s * 512: s * 512 + nt] = hvals[s]
            hi_idx[:, hi_t * 32 * s_hi:(hi_t + 1) * 32 * s_hi] = (
                hflat.reshape(-1, 16).T)
            hi_t += 1
        else:
            assert (sl_h == 0).all(), t
        off += nt
    assert off == nsh
    return dict(lo_idx=np.tile(lo_idx, (8, 1)),
                pr_idx=np.tile(pr_idx, (8, 1)),
                hi_idx=np.tile(hi_idx, (8, 1)))


# --------------------------------------------------------------------------
# device program
# --------------------------------------------------------------------------

def _build_program(nsh, tiles, lo_cap, hi_cap, s_hi, planes_lo, has_hi,
                   planes_pair):
    dt = mybir.dt
    nc = bacc.Bacc("TRN2", target_bir_lowering=False, num_swdge_queues=N_QUEUES)
    n_ids = lo_cap + hi_cap
    K = DEG_K

    lo_cols = sum(nt * (pl - 2 * pp) // 16
                  for nt, pl, pp in zip(tiles, planes_lo, planes_pair))
    pr_cols = sum(nt * pp // 16 for nt, pp in zip(tiles, planes_pair))
    n_hi_tiles = int(sum(has_hi))

    table_d = nc.dram_tensor("table", [n_ids, C], dt.bfloat16, kind="ExternalInput")
    tablep_d = nc.dram_tensor("tablep", [n_ids // 2, 2 * C], dt.bfloat16,
                              kind="ExternalInput")
    lo_idx_d = nc.dram_tensor("lo_idx", [P, max(lo_cols, 16)], dt.int16,
                              kind="ExternalInput")
    pr_idx_d = nc.dram_tensor("pr_idx", [P, max(pr_cols, 16)], dt.int16,
                              kind="ExternalInput")
    hi_idx_d = nc.dram_tensor("hi_idx", [P, 32 * s_hi * max(n_hi_tiles, 1)],
                              dt.int16, kind="ExternalInput")
    x0t_d = nc.dram_tensor("x0t", [P, nsh], dt.bfloat16, kind="ExternalInput")
    xself_d = nc.dram_tensor("xself", [P, nsh], dt.bfloat16, kind="ExternalInput")
    m1t_d = nc.dram_tensor("m1t", [P, C], dt.bfloat16, kind="ExternalInput")
    m2t_d = nc.dram_tensor("m2t", [P, C], dt.bfloat16, kind="ExternalInput")
    bias_d = nc.dram_tensor("biasv", [P, 1], dt.float32, kind="ExternalInput")
    ident_d = nc.dram_tensor("ident", [P, P], dt.bfloat16, kind="ExternalInput")
    out_d = nc.dram_tensor("out", [P, nsh], dt.bfloat16, kind="ExternalOutput")

    with TileContext(nc) as tc:
        with (
            tc.tile_pool(name="consts", bufs=1) as cpool,
            tc.tile_pool(name="work", bufs=WORK_BUFS) as pool,
            tc.tile_pool(name="gpool", bufs=G_BUFS) as gpool,
            tc.tile_pool(name="psum", bufs=PSUM_BUFS, space="PSUM") as ppool,
        ):
            # issue tile-0 index loads before the consts so the first gather
            # starts as early as possible
            ncol0 = tiles[0] * (planes_lo[0] - 2 * planes_pair[0]) // 16
            pcol0 = tiles[0] * planes_pair[0] // 16
            pr_i0 = None
            if pcol0:
                pr_i0 = pool.tile([P, pcol0], dt.int16)
                nc.sync.dma_start(out=pr_i0[:], in_=pr_idx_d[:, 0:pcol0])
            lo_i0 = pool.tile([P, ncol0], dt.int16)
            nc.sync.dma_start(out=lo_i0[:], in_=lo_idx_d[:, 0:ncol0])
            hi_i0 = None
            if has_hi[0]:
                hi_i0 = pool.tile([P, 32 * s_hi], dt.int16)
                nc.sync.dma_start(out=hi_i0[:], in_=hi_idx_d[:, 0:32 * s_hi])

            m1t = cpool.tile([P, C], dt.bfloat16)
            nc.sync.dma_start(out=m1t[:], in_=m1t_d[:])
            m2t = cpool.tile([P, C], dt.bfloat16)
            nc.sync.dma_start(out=m2t[:], in_=m2t_d[:])
            biasv = cpool.tile([P, 1], dt.float32)
            nc.sync.dma_start(out=biasv[:], in_=bias_d[:])
            ident = cpool.tile([P, P], dt.bfloat16)
            nc.sync.dma_start(out=ident[:], in_=ident_d[:])

            n_tiles = len(tiles)
            # tail tiles: gathers prefetched early into persistent buffers so
            # only their (small) compute chain is exposed at kernel end
            tail_from = n_tiles - 2 if (TAIL_PREFETCH and n_tiles >= 3) else n_tiles
            tile_cols = [nt * (planes_lo[t] - 2 * planes_pair[t]) // 16
                         for t, nt in enumerate(tiles)]
            tile_pcols = [nt * planes_pair[t] // 16 for t, nt in enumerate(tiles)]
            tile_off = np.cumsum([0] + list(tiles))
            col_off = np.cumsum([0] + tile_cols)
            pcol_off = np.cumsum([0] + tile_pcols)
            hi_num = np.cumsum([0] + [int(h) for h in has_hi])

            def issue_gather(lo_i, hi_i, pr_i, g_lo, g_hi, g_pr, n_lo, n_pp,
                             nt, use_hi, ramp=False):
                for k in range(n_pp):
                    nc.gpsimd.dma_gather(
                        out_ap=g_pr[:, k, :, :],
                        in_ap=tablep_d[0:16384, :],
                        idxs_ap=pr_i[:, k * nt // 16:(k + 1) * nt // 16],
                        num_idxs=nt, num_idxs_reg=idx_reg(nt), elem_size=2 * C,
                        transpose=True, queue_num=qctr[0] % N_QUEUES,
                        single_packet=SINGLE_PACKET)
                    qctr[0] += 1
                CH = GATHER_CHUNK
                ramp_sizes = [128, 256, 512] if ramp and RAMP_FIRST else []
                c0 = 0
                while c0 < n_lo:
                    cn = min(ramp_sizes.pop(0) if ramp_sizes else CH, n_lo - c0)
                    nc.gpsimd.dma_gather(
                        out_ap=g_lo[:, :, c0:c0 + cn],
                        in_ap=table_d[0:lo_cap, :],
                        idxs_ap=lo_i[:, c0 // 16:(c0 + cn) // 16],
                        num_idxs=cn, num_idxs_reg=idx_reg(cn), elem_size=C,
                        transpose=True, queue_num=qctr[0] % N_QUEUES,
                        single_packet=SINGLE_PACKET)
                    qctr[0] += 1
                    c0 += cn
                if use_hi:
                    n_hi = s_hi * 512
                    c0 = 0
                    while c0 < n_hi:
                        cn = min(CH, n_hi - c0)
                        nc.gpsimd.dma_gather(
                            out_ap=g_hi[:, :, c0:c0 + cn],
                            in_ap=table_d[lo_cap:, :],
                            idxs_ap=hi_i[:, c0 // 16:(c0 + cn) // 16],
                            num_idxs=cn, num_idxs_reg=idx_reg(cn), elem_size=C,
                            transpose=True, queue_num=qctr[0] % N_QUEUES,
                            single_packet=SINGLE_PACKET)
                        qctr[0] += 1
                        c0 += cn

            def load_idx(t, first=False):
                ncol = tile_cols[t]
                if first:
                    lo_i, hi_i, pr_i = lo_i0, hi_i0, pr_i0
                else:
                    lo_i = pool.tile([P, ncol], dt.int16)
                    nc.sync.dma_start(
                        out=lo_i[:],
                        in_=lo_idx_d[:, col_off[t]:col_off[t] + ncol])
                    hi_i = None
                    if has_hi[t]:
                        h = hi_num[t]
                        hi_i = pool.tile([P, 32 * s_hi], dt.int16)
                        nc.sync.dma_start(
                            out=hi_i[:],
                            in_=hi_idx_d[:, h * 32 * s_hi:(h + 1) * 32 * s_hi])
                    pr_i = None
                    if tile_pcols[t]:
                        pr_i = pool.tile([P, tile_pcols[t]], dt.int16)
                        nc.sync.dma_start(
                            out=pr_i[:],
                            in_=pr_idx_d[:, pcol_off[t]:pcol_off[t] + tile_pcols[t]])
                return lo_i, hi_i, pr_i

            # tile 0 gathers first (lead-in), then tail-tile gathers (prepaid)
            g_cache = {}
            lo_i, hi_i = load_idx(0, first=True)
            g_lo = gpool.tile([P, 1, planes_lo[0] * tiles[0]], dt.bfloat16,
                              name="g_lo")
            g_hi = (gpool.tile([P, 1, s_hi * 512], dt.bfloat16, name="g_hi")
                    if has_hi[0] else None)
            issue_gather(lo_i, hi_i, g_lo, g_hi, planes_lo[0] * tiles[0], has_hi[0])
            g_cache[0] = (g_lo, g_hi)

            for t in range(tail_from, n_tiles):
                nt = tiles[t]
                assert not has_hi[t]
                lo_i, _ = load_idx(t)
                g_lo = cpool.tile([P, 1, planes_lo[t] * nt], dt.bfloat16,
                                  name=f"g_tail{t}")
                issue_gather(lo_i, None, g_lo, None, planes_lo[t] * nt, False)
                g_cache[t] = (g_lo, None)

            for t, nt in enumerate(tiles):
                n0 = tile_off[t]
                pl = planes_lo[t]

                if t in g_cache:
                    g_lo, g_hi = g_cache.pop(t)
                else:
                    lo_i, hi_i = load_idx(t)
                    g_lo = gpool.tile([P, 1, pl * nt], dt.bfloat16,
                                      name="g_lo")
                    g_hi = (gpool.tile([P, 1, s_hi * 512], dt.bfloat16,
                                       name="g_hi")
                            if has_hi[t] else None)
                    issue_gather(lo_i, hi_i, g_lo, g_hi, pl * nt, has_hi[t])

                psum_a = ppool.tile([P, nt], dt.float32)
                n_planes = pl + (s_hi if has_hi[t] else 0)
                pi = 0
                for s in range(pl):
                    nc.tensor.matmul(
                        psum_a[:], lhsT=ident[:],
                        rhs=g_lo[:, 0, s * nt:(s + 1) * nt],
                        start=(pi == 0), stop=(pi == n_planes - 1))
                    pi += 1
                if has_hi[t]:
                    for s in range(s_hi):
                        nc.tensor.matmul(
                            psum_a[:], lhsT=ident[:],
                            rhs=g_hi[:, 0, s * 512:s * 512 + nt],
                            start=(pi == 0), stop=(pi == n_planes - 1))
                        pi += 1

                gsum = pool.tile([P, nt], dt.bfloat16)
                nc.vector.tensor_copy(out=gsum[:], in_=psum_a[:])

                x0_t = pool.tile([P, nt], dt.bfloat16)
                nc.sync.dma_start(out=x0_t[:], in_=x0t_d[:, n0:n0 + nt])
                xs_t = pool.tile([P, nt], dt.bfloat16)
                nc.sync.dma_start(out=xs_t[:], in_=xself_d[:, n0:n0 + nt])

                psum_b = ppool.tile([P, nt], dt.float32)
                nc.tensor.matmul(psum_b[:], lhsT=m1t[:], rhs=gsum[:],
                                 start=True, stop=False)
                nc.tensor.matmul(psum_b[:], lhsT=m1t[:], rhs=xs_t[:],
                                 start=False, stop=False)
                nc.tensor.matmul(psum_b[:], lhsT=m2t[:], rhs=x0_t[:],
                                 start=False, stop=True)

                out_t = pool.tile([P, nt], dt.bfloat16)
                nc.scalar.activation(
                    out_t[:], psum_b[:], mybir.ActivationFunctionType.Relu,
                    bias=biasv[:, 0:1], scale=1.0)
                nc.sync.dma_start(out=out_d[:, n0:n0 + nt], in_=out_t[:])
    nc.compile()
    return nc


# --------------------------------------------------------------------------
# full host prep (shared by kernel() and tests)
# --------------------------------------------------------------------------

def _prepare(x, x_0, edge_index, W1, W2, bias, n_cores, lo_cap, s_hi_try=(1, 2, 3, 4, 6, 8)):
    x = np.asarray(x, dtype=F32)          # [1, C, N, 1]
    x_0 = np.asarray(x_0, dtype=F32)      # [1, N, C]
    ei = np.asarray(edge_index)           # [2, 1, N, K]
    W1 = np.asarray(W1, dtype=F32)
    W2 = np.asarray(W2, dtype=F32)
    bias = np.asarray(bias, dtype=F32)

    n_rows = x.shape[2]
    nsh = n_rows // n_cores
    idx_all = np.asarray(ei[0, 0], dtype=np.int64)   # [N, K]
    K = idx_all.shape[1]
    assert K == DEG_K

    x_cn = np.ascontiguousarray(x[0, :, :, 0])       # [C, N]
    x_nm = np.ascontiguousarray(x_cn.T)              # [N, C]
    x_bf16 = x_nm.astype(BF16)
    x0_cn = np.ascontiguousarray(x_0[0].T)           # [C, N]

    deg = K + 1
    s1 = (1.0 - ALPHA) * (1.0 - BETA)
    s2 = ALPHA * (1.0 - BETA)
    eye = np.eye(C, dtype=np.float64)
    m1sT = ((s1 * eye + BETA * W1.astype(np.float64)).T / deg).astype(BF16)
    m2T = ((s2 * eye + BETA * W2.astype(np.float64)).T).astype(BF16)
    bias_v = np.ascontiguousarray(bias.reshape(-1)[:, None].astype(F32))
    ident = np.eye(P, dtype=BF16)

    tiles = _split_tiles(nsh)
    hi_needed = n_rows - (lo_cap - 1)
    hi_cap = 0
    if hi_needed > 0:
        hi_cap = ((hi_needed + 1 + P - 1) // P) * P

    infos = None
    s_hi_used = None
    for s_hi in s_hi_try:
        infos = []
        ok = True
        for c in range(n_cores):
            sl = slice(c * nsh, (c + 1) * nsh)
            info = _core_hi_info(x_bf16, idx_all[sl], nsh, n_rows, lo_cap, s_hi)
            if info is None:
                ok = False
                break
            infos.append(info)
        if ok:
            s_hi_used = s_hi
            break
    assert s_hi_used is not None, "could not find feasible s_hi"
    s_hi = s_hi_used

    # shared tile structure across cores (SPMD: one program for all).
    # hi-node block occupies node positions [head_lo, head_lo + hi_count_c)
    hi_counts = [info["n_hi_nodes"] for info in infos]
    head_lo = nsh % 512 if SMALL_FIRST else 0
    cum = np.cumsum([0] + tiles)
    planes_lo = []
    has_hi = []
    for t, nt in enumerate(tiles):
        pure = (cum[t] >= head_lo and cum[t + 1] <= head_lo + min(hi_counts)
                and nt == 512)
        hi = cum[t + 1] > head_lo and cum[t] < head_lo + max(hi_counts)
        planes_lo.append(DEG_K - 1 if pure else DEG_K)
        has_hi.append(bool(hi))
    # hi tiles must be full 512 tiles (hi grid planes are 512 wide)
    for t, h in enumerate(has_hi):
        if h:
            assert tiles[t] == 512, (tiles, has_hi)

    # pair planes per tile: limited by the weakest node in the tile across
    # all cores (pair gathers need num_idxs % 128 == 0 -> 512-node tiles)
    planes_pair = []
    for t, nt in enumerate(tiles):
        if nt != 512:
            planes_pair.append(0)
            continue
        pmin = min(int(info["p_cnt"][info["perm"]][cum[t]:cum[t + 1]].min())
                   for info in infos)
        planes_pair.append(min(pmin, planes_lo[t] // 2))

    in_maps = []
    perms = []
    for c in range(n_cores):
        sl = slice(c * nsh, (c + 1) * nsh)
        info = infos[c]
        perm = info["perm"]
        perms.append(perm)
        d = _pack_core(info, nsh, lo_cap, hi_cap, s_hi, tiles, planes_lo,
                       has_hi, planes_pair)
        n_ids = lo_cap + hi_cap
        table = np.zeros((n_ids, C), dtype=BF16)
        table[info["lid"]] = x_bf16
        gsl = np.arange(c * nsh, (c + 1) * nsh)[perm]  # global node ids, perm order
        in_maps.append(dict(
            table=table,
            tablep=np.ascontiguousarray(table.reshape(n_ids // 2, 2 * C)),
            lo_idx=d["lo_idx"],
            pr_idx=d["pr_idx"],
            hi_idx=d["hi_idx"],
            x0t=np.ascontiguousarray(x0_cn[:, gsl]).astype(BF16),
            xself=np.ascontiguousarray(x_cn[:, gsl]).astype(BF16),
            m1t=m1sT,
            m2t=m2T,
            biasv=bias_v,
            ident=ident,
        ))
    meta = dict(nsh=nsh, tiles=tiles, lo_cap=lo_cap, hi_cap=hi_cap,
                s_hi=s_hi, n_rows=n_rows, planes_lo=planes_lo, has_hi=has_hi,
                planes_pair=planes_pair, perms=perms)
    return in_maps, meta


last_results = None  # BassKernelResults of the most recent kernel() call


def kernel(x, x_0, edge_index, W1, W2, bias):
    global last_results
    import os
    in_maps, meta = _prepare(x, x_0, edge_index, W1, W2, bias,
                             n_cores=N_CORES, lo_cap=LO_CAP_FULL)
    nc = _build_program(meta["nsh"], meta["tiles"], meta["lo_cap"],
                        meta["hi_cap"], meta["s_hi"], meta["planes_lo"],
                        meta["has_hi"], meta["planes_pair"])
    trace = os.environ.get("GCNII_TRACE", "") == "1"
    res = run_bass_kernel_spmd(nc, in_maps, core_ids=list(range(N_CORES)),
                               trace=trace)
    last_results = res
    outs = []
    for c, r in enumerate(res.results):
        o = np.empty_like(r["out"])
        o[:, meta["perms"][c]] = r["out"]   # undo node permutation
        outs.append(o)
    out = np.concatenate(outs, axis=1)
    return np.ascontiguousarray(out.astype(F32))[None, :, :, None]


# revision 4
# speedup vs baseline: 1.0302x; 1.0302x over previous
"""GCNII conv (gnn_message_passing) Trainium2 Bass kernel.

Strategy (8-way node sharding, DRAM-resident relabeled feature table):
  - Host: relabel node-rows per core so gather indices fit int16 windows:
    "lo" window = table rows [0, 32768) (row 0 zeros), "hi" window = rows
    [32768, 40960) (row 32768 zeros).  Rows are assigned so each node has at
    most s_hi refs into the hi window.  Nodes with a hi ref are permuted to
    the FRONT of the core's shard so only the leading tiles need a hi gather
    plane; tiles made purely of hi-nodes drop their all-padding 16th lo
    plane.  The inverse permutation is applied to the output on host.
  - Device: dma_gather in transpose mode sources 256B rows directly from the
    DRAM table (no SBUF table copy); PE sums the neighbor slots plus self via
    bf16 identity matmuls into PSUM (fp32 exact), then the GCNII combine is
    bf16 GEMMs (M1s = (s1*I + beta*W1)/deg on gather_sum + x_self, M2 =
    s2*I + beta*W2 on x_0) plus bias+ReLU on the activation engine.
"""

import numpy as np
import ml_dtypes

import concourse.bacc as bacc
import concourse.mybir as mybir
from concourse.tile import TileContext
from concourse.bass_utils import run_bass_kernel_spmd

BF16 = ml_dtypes.bfloat16
F32 = np.float32

ALPHA = 0.1
BETA = float(np.log(0.5 / 4 + 1.0))
DEG_K = 16           # neighbors per node (w/o self loop)
C = 128              # channels
P = 128              # partitions

N_FULL = 40000
N_CORES = 8
LO_CAP_FULL = 32768  # rows in lo window (incl zero row at local id 0)

GATHER_CHUNK = 896   # idxs per dma_gather instruction (ucode cap ~992, %128)
SINGLE_PACKET = True
TAIL_PREFETCH = False  # issue tail-tile gathers early (helps only if DMA slack)
SPLIT_TAIL = True      # split the last tile to shorten the exposed end chain
SMALL_FIRST = False    # head bubbles block the critical path; keep small tiles last
RAMP_FIRST = False     # progressive tile-0 chunks regressed in sim
SHARED_REGS = True     # one num_idxs register per distinct chunk size


# --------------------------------------------------------------------------
# host-side preparation
# --------------------------------------------------------------------------

def _choose_hi_rows(refs, owners, n_rows, nsh, hi_needed, s_hi):
    """Pick `hi_needed` rows for the hi window s.t. no node has more than
    `s_hi` references into the hi window.  Prefers cold rows."""
    counts = np.bincount(refs, minlength=n_rows)
    order = np.argsort(counts, kind="stable")
    si = np.argsort(refs, kind="stable")
    owners_s = owners[si]
    starts = np.searchsorted(refs[si], np.arange(n_rows))
    ends = np.searchsorted(refs[si], np.arange(n_rows) + 1)

    is_hi = np.zeros(n_rows, dtype=bool)
    node_cnt = np.zeros(nsh, dtype=np.int64)
    zero_rows = order[counts[order] == 0]
    take = zero_rows[: min(len(zero_rows), hi_needed)]
    is_hi[take] = True
    n_hi = len(take)
    if n_hi < hi_needed:
        for r in order:
            if counts[r] == 0 or is_hi[r]:
                continue
            ow = owners_s[starts[r]:ends[r]]
            u, m = np.unique(ow, return_counts=True)
            if (node_cnt[u] + m <= s_hi).all():
                node_cnt[u] += m
                is_hi[r] = True
                n_hi += 1
                if n_hi == hi_needed:
                    break
    if n_hi != hi_needed:
        return None
    return is_hi


def _choose_hi_rows_masked(refs, owners, n_rows, nsh, hi_needed, s_hi, counts):
    """_choose_hi_rows with externally supplied counts (paired rows masked)."""
    order = np.argsort(counts, kind="stable")
    si = np.argsort(refs, kind="stable")
    owners_s = owners[si]
    starts = np.searchsorted(refs[si], np.arange(n_rows))
    ends = np.searchsorted(refs[si], np.arange(n_rows) + 1)

    is_hi = np.zeros(n_rows, dtype=bool)
    node_cnt = np.zeros(nsh, dtype=np.int64)
    zero_rows = order[counts[order] == 0]
    take = zero_rows[: min(len(zero_rows), hi_needed)]
    is_hi[take] = True
    n_hi = len(take)
    if n_hi < hi_needed:
        for r in order:
            if counts[r] == 0 or counts[r] >= (1 << 29) or is_hi[r]:
                continue
            ow = owners_s[starts[r]:ends[r]]
            u, m = np.unique(ow, return_counts=True)
            if (node_cnt[u] + m <= s_hi).all():
                node_cnt[u] += m
                is_hi[r] = True
                n_hi += 1
                if n_hi == hi_needed:
                    break
    if n_hi != hi_needed:
        return None
    return is_hi


def _split_tiles(nsh):
    """Tile sizes; small tail tile shortens the exposed end-of-kernel chain."""
    n_full, left = divmod(nsh, 512)
    if left == 0:
        small = []
    elif SPLIT_TAIL and left > 128 and left % 8 == 0 and (left - 128) % 8 == 0 \
            and left - 128 >= 8:
        small = [128, left - 128] if SMALL_FIRST else [left - 128, 128]
    else:
        small = [left]
    tiles = small + [512] * n_full if SMALL_FIRST else [512] * n_full + small
    assert sum(tiles) == nsh
    return tiles


def _greedy_match(refs_n, n_rows, eligible):
    """Greedy matching on the co-reference graph restricted to eligible rows:
    pair rows that some node references together.  partner[r] = row or -1."""
    nsh, K = refs_n.shape
    us, vs = [], []
    for a in range(K):
        for b in range(a + 1, K):
            u = refs_n[:, a]
            v = refs_n[:, b]
            m = (u != v) & eligible[u] & eligible[v]
            uu = np.minimum(u[m], v[m])
            vv = np.maximum(u[m], v[m])
            us.append(uu)
            vs.append(vv)
    U = np.concatenate(us)
    V = np.concatenate(vs)
    partner = np.full(n_rows, -1, dtype=np.int64)
    rng = np.random.default_rng(0)
    for _ in range(12):
        free = (partner[U] < 0) & (partner[V] < 0)
        U, V = U[free], V[free]
        if len(U) == 0:
            break
        sh = rng.permutation(len(U))
        U, V = U[sh], V[sh]
        o = np.argsort(U, kind="stable")
        U, V = U[o], V[o]
        fu = np.ones(len(U), bool)
        fu[1:] = U[1:] != U[:-1]
        U1, V1 = U[fu], V[fu]
        o2 = np.argsort(V1, kind="stable")
        U1, V1 = U1[o2], V1[o2]
        fv = np.ones(len(V1), bool)
        fv[1:] = V1[1:] != V1[:-1]
        U2, V2 = U1[fv], V1[fv]
        ok = ~np.isin(U2, V2) & ~np.isin(V2, U2)
        partner[U2[ok]] = V2[ok]
        partner[V2[ok]] = U2[ok]
    return partner


def _core_hi_info(x_bf16, idx_shard, nsh, n_rows, lo_cap, s_hi):
    """Phase 1: hi rows, row pairing among lo rows, node permutation."""
    K = idx_shard.shape[1]
    refs_n = idx_shard.astype(np.int64)              # [nsh, K]
    refs = refs_n.reshape(-1)
    owners = np.repeat(np.arange(nsh, dtype=np.int64), K)
    # ids: [zero pair (2)] [paired rows] [unpaired lo rows] | [hi rows]
    hi_needed = max(0, n_rows + 2 - lo_cap)
    if hi_needed > 0:
        is_hi = _choose_hi_rows(refs, owners, n_rows, nsh, hi_needed, s_hi)
        if is_hi is None:
            return None
    else:
        is_hi = np.zeros(n_rows, dtype=bool)

    partner = _greedy_match(refs_n, n_rows, ~is_hi)

    # per-node pair selection over ref occurrences (each occurrence used once)
    used = np.zeros((nsh, K), dtype=bool)
    p_cnt = np.zeros(nsh, dtype=np.int64)
    for a in range(K):
        for b in range(a + 1, K):
            u = refs_n[:, a]
            v = refs_n[:, b]
            cand = (~used[:, a] & ~used[:, b] & (partner[u] == v)
                    & (u != v) & (p_cnt < K // 2))
            if not cand.any():
                continue
            used[cand, a] = True
            used[cand, b] = True
            p_cnt[cand] += 1

    # rows actually pair-fetched: keep ALL matched row pairs in the pair
    # region (a matched row might also be single-fetched; its id stays
    # < 2*(n_pairs+1) < lo_cap so the lo singles window reaches it)
    matched = partner >= 0
    plist = np.where(matched & (partner > np.arange(n_rows)))[0]
    n_pairs = len(plist)
    pair_id = np.full(n_rows, -1, dtype=np.int64)
    pair_id[plist] = 1 + np.arange(n_pairs)          # pair 0 = zero pad pair
    pair_id[partner[plist]] = pair_id[plist]

    lid = np.empty(n_rows, dtype=np.int64)
    lid[plist] = 2 * pair_id[plist]
    lid[partner[plist]] = 2 * pair_id[plist] + 1
    unp_lo = np.where(~matched & ~is_hi)[0]
    base = 2 * (n_pairs + 1)
    lid[unp_lo] = base + np.arange(len(unp_lo))
    hi_rows = np.where(is_hi)[0]
    lid[hi_rows] = lo_cap + 1 + np.arange(len(hi_rows))
    assert base + len(unp_lo) <= lo_cap, (base, len(unp_lo))
    assert n_pairs + 1 <= 16384, n_pairs

    lids_n = lid[refs_n]
    h_n = (lids_n >= lo_cap).sum(axis=1)
    # node order: hi-nodes first, then rest; within each group pair-count desc
    key = (h_n == 0).astype(np.int64) * (K + 2) + (K // 2 - p_cnt)
    perm = np.argsort(key, kind="stable")
    return dict(lid=lid, lids_n=lids_n, h_n=h_n, perm=perm,
                n_hi_nodes=int((h_n > 0).sum()), p_cnt=p_cnt,
                pair_id=pair_id, used=used, refs_n=refs_n,
                n_pairs=n_pairs)


def _pack_core(info, nsh, lo_cap, hi_cap, s_hi, tiles, planes_lo, has_hi,
               planes_pair):
    """Phase 2: build index grids given the shared tile structure."""
    lid = info["lid"]
    perm = info["perm"]
    refs_n = info["refs_n"][perm]        # [nsh, K] global rows, permuted
    used = info["used"][perm]
    h_n = info["h_n"][perm]
    p_cnt = info["p_cnt"][perm]
    pair_id = info["pair_id"]
    K = refs_n.shape[1]
    lids_n = lid[refs_n]

    lo_cols = sum(nt * (pl - 2 * pp) // 16
                  for nt, pl, pp in zip(tiles, planes_lo, planes_pair))
    pr_cols = sum(nt * pp // 16 for nt, pp in zip(tiles, planes_pair))
    n_hi_tiles = int(sum(has_hi))
    lo_idx = np.zeros((16, max(lo_cols, 16)), dtype=np.int16)
    pr_idx = np.zeros((16, max(pr_cols, 16)), dtype=np.int16)
    hi_idx = np.zeros((16, 32 * s_hi * max(n_hi_tiles, 1)), dtype=np.int16)
    off = 0
    lcol = 0
    pcol = 0
    hi_t = 0
    for t, nt in enumerate(tiles):
        pl = planes_lo[t]
        pp = planes_pair[t]
        sl_r = refs_n[off:off + nt]
        sl_u = used[off:off + nt].copy()
        sl_h = h_n[off:off + nt]
        assert (p_cnt[off:off + nt] >= pp).all(), (t, pp)
        # pair slots: first pp pairs of each node -> pair ids
        pvals = np.zeros((nt, pp), dtype=np.int64)
        consumed = np.zeros_like(sl_u)
        if pp:
            taken = np.zeros(nt, dtype=np.int64)
            for a in range(K):
                for b in range(a + 1, K):
                    u = sl_r[:, a]
                    v = sl_r[:, b]
                    cand = (sl_u[:, a] & sl_u[:, b]
                            & ~consumed[:, a] & ~consumed[:, b]
                            & (pair_id[u] >= 0)
                            & (pair_id[u] == pair_id[v]) & (u != v)
                            & (taken < pp))
                    if not cand.any():
                        continue
                    pvals[cand, taken[cand]] = pair_id[u[cand]]
                    consumed[cand, a] = True
                    consumed[cand, b] = True
                    taken[cand] += 1
            assert (taken == pp).all(), (t, pp, taken.min())
            flatp = pvals.T.reshape(-1)
            ncolp = pp * nt // 16
            pr_idx[:, pcol:pcol + ncolp] = flatp.reshape(-1, 16).T
            pcol += ncolp
        # singles: all lo-ref occurrences not consumed as pairs (includes
        # never-paired occurrences and leftover unused pairs)
        n_sing = pl - 2 * pp
        svals = np.zeros((nt, n_sing), dtype=np.int64)
        islo = lids_n[off:off + nt] < lo_cap
        for j_col in range(nt):
            occ = np.where(~consumed[j_col] & islo[j_col])[0]
            vals = lids_n[off + j_col, occ]
            assert len(vals) <= n_sing, (t, j_col, len(vals), n_sing)
            svals[j_col, :len(vals)] = vals
        flat = svals.T.reshape(-1)
        ncol = n_sing * nt // 16
        lo_idx[:, lcol:lcol + ncol] = flat.reshape(-1, 16).T
        lcol += ncol
        # hi plane
        if has_hi[t]:
            assert nt == 512
            hvals = np.zeros((s_hi, nt), dtype=np.int64)
            ishi_n = lids_n[off:off + nt] >= lo_cap
            for j_col in range(nt):
                occ = np.where(ishi_n[j_col])[0]
                assert len(occ) <= s_hi
                for s, o in enumerate(occ):
                    hvals[s, j_col] = lids_n[off + j_col, o] - lo_cap
            hflat = np.zeros(s_hi * 512, dtype=np.int64)
            for s in range(s_hi):
                hflat[s * 512: s * 512 + nt] = hvals[s]
            hi_idx[:, hi_t * 32 * s_hi:(hi_t + 1) * 32 * s_hi] = (
                hflat.reshape(-1, 16).T)
            hi_t += 1
        else:
            assert (sl_h == 0).all(), t
        off += nt
    assert off == nsh
    return dict(lo_idx=np.tile(lo_idx, (8, 1)),
                pr_idx=np.tile(pr_idx, (8, 1)),
                hi_idx=np.tile(hi_idx, (8, 1)))


# --------------------------------------------------------------------------
# device program
# --------------------------------------------------------------------------

def _build_program(nsh, tiles, lo_cap, hi_cap, s_hi, planes_lo, has_hi,
                   planes_pair):
    dt = mybir.dt
    nc = bacc.Bacc("TRN2", target_bir_lowering=False, num_swdge_queues=N_QUEUES)
    n_ids = lo_cap + hi_cap
    K = DEG_K

    lo_cols = sum(nt * (pl - 2 * pp) // 16
                  for nt, pl, pp in zip(tiles, planes_lo, planes_pair))
    pr_cols = sum(nt * pp // 16 for nt, pp in zip(tiles, planes_pair))
    n_hi_tiles = int(sum(has_hi))

    table_d = nc.dram_tensor("table", [n_ids, C], dt.bfloat16, kind="ExternalInput")
    tablep_d = nc.dram_tensor("tablep", [n_ids // 2, 2 * C], dt.bfloat16,
                              kind="ExternalInput")
    lo_idx_d = nc.dram_tensor("lo_idx", [P, max(lo_cols, 16)], dt.int16,
                              kind="ExternalInput")
    pr_idx_d = nc.dram_tensor("pr_idx", [P, max(pr_cols, 16)], dt.int16,
                              kind="ExternalInput")
    hi_idx_d = nc.dram_tensor("hi_idx", [P, 32 * s_hi * max(n_hi_tiles, 1)],
                              dt.int16, kind="ExternalInput")
    x0t_d = nc.dram_tensor("x0t", [P, nsh], dt.bfloat16, kind="ExternalInput")
    xself_d = nc.dram_tensor("xself", [P, nsh], dt.bfloat16, kind="ExternalInput")
    m1t_d = nc.dram_tensor("m1t", [P, C], dt.bfloat16, kind="ExternalInput")
    m2t_d = nc.dram_tensor("m2t", [P, C], dt.bfloat16, kind="ExternalInput")
    bias_d = nc.dram_tensor("biasv", [P, 1], dt.float32, kind="ExternalInput")
    ident_d = nc.dram_tensor("ident", [P, P], dt.bfloat16, kind="ExternalInput")
    out_d = nc.dram_tensor("out", [P, nsh], dt.bfloat16, kind="ExternalOutput")

    with TileContext(nc) as tc:
        with (
            tc.tile_pool(name="consts", bufs=1) as cpool,
            tc.tile_pool(name="work", bufs=WORK_BUFS) as pool,
            tc.tile_pool(name="gpool", bufs=G_BUFS) as gpool,
            tc.tile_pool(name="psum", bufs=PSUM_BUFS, space="PSUM") as ppool,
        ):
            # issue tile-0 index loads before the consts so the first gather
            # starts as early as possible
            ncol0 = tiles[0] * (planes_lo[0] - 2 * planes_pair[0]) // 16
            pcol0 = tiles[0] * planes_pair[0] // 16
            pr_i0 = None
            if pcol0:
                pr_i0 = pool.tile([P, pcol0], dt.int16)
                nc.sync.dma_start(out=pr_i0[:], in_=pr_idx_d[:, 0:pcol0])
            lo_i0 = pool.tile([P, ncol0], dt.int16)
            nc.sync.dma_start(out=lo_i0[:], in_=lo_idx_d[:, 0:ncol0])
            hi_i0 = None
            if has_hi[0]:
                hi_i0 = pool.tile([P, 32 * s_hi], dt.int16)
                nc.sync.dma_start(out=hi_i0[:], in_=hi_idx_d[:, 0:32 * s_hi])

            m1t = cpool.tile([P, C], dt.bfloat16)
            nc.sync.dma_start(out=m1t[:], in_=m1t_d[:])
            m2t = cpool.tile([P, C], dt.bfloat16)
            nc.sync.dma_start(out=m2t[:], in_=m2t_d[:])
            biasv = cpool.tile([P, 1], dt.float32)
            nc.sync.dma_start(out=biasv[:], in_=bias_d[:])
            ident = cpool.tile([P, P], dt.bfloat16)
            nc.sync.dma_start(out=ident[:], in_=ident_d[:])

            n_tiles = len(tiles)
            # tail tiles: gathers prefetched early into persistent buffers so
            # only their (small) compute chain is exposed at kernel end
            tail_from = n_tiles - 2 if (TAIL_PREFETCH and n_tiles >= 3) else n_tiles
            tile_cols = [nt * (planes_lo[t] - 2 * planes_pair[t]) // 16
                         for t, nt in enumerate(tiles)]
            tile_pcols = [nt * planes_pair[t] // 16 for t, nt in enumerate(tiles)]
            tile_off = np.cumsum([0] + list(tiles))
            col_off = np.cumsum([0] + tile_cols)
            pcol_off = np.cumsum([0] + tile_pcols)
            hi_num = np.cumsum([0] + [int(h) for h in has_hi])

            def issue_gather(lo_i, hi_i, pr_i, g_lo, g_hi, g_pr, n_lo, n_pp,
                             nt, use_hi, ramp=False):
                if n_pp:
                    npr = n_pp * nt
                    nc.gpsimd.dma_gather(
                        out_ap=g_pr[:],
                        in_ap=tablep_d[0:16384, :],
                        idxs_ap=pr_i[:, 0:npr // 16],
                        num_idxs=npr, num_idxs_reg=idx_reg(npr),
                        elem_size=2 * C, transpose=True,
                        queue_num=qctr[0] % N_QUEUES,
                        single_packet=SINGLE_PACKET and npr <= GATHER_CHUNK)
                    qctr[0] += 1
                CH = GATHER_CHUNK
                ramp_sizes = [128, 256, 512] if ramp and RAMP_FIRST else []
                c0 = 0
                while c0 < n_lo:
                    cn = min(ramp_sizes.pop(0) if ramp_sizes else CH, n_lo - c0)
                    nc.gpsimd.dma_gather(
                        out_ap=g_lo[:, :, c0:c0 + cn],
                        in_ap=table_d[0:lo_cap, :],
                        idxs_ap=lo_i[:, c0 // 16:(c0 + cn) // 16],
                        num_idxs=cn, num_idxs_reg=idx_reg(cn), elem_size=C,
                        transpose=True, queue_num=qctr[0] % N_QUEUES,
                        single_packet=SINGLE_PACKET)
                    qctr[0] += 1
                    c0 += cn
                if use_hi:
                    n_hi = s_hi * 512
                    c0 = 0
                    while c0 < n_hi:
                        cn = min(CH, n_hi - c0)
                        nc.gpsimd.dma_gather(
                            out_ap=g_hi[:, :, c0:c0 + cn],
                            in_ap=table_d[lo_cap:, :],
                            idxs_ap=hi_i[:, c0 // 16:(c0 + cn) // 16],
                            num_idxs=cn, num_idxs_reg=idx_reg(cn), elem_size=C,
                            transpose=True, queue_num=qctr[0] % N_QUEUES,
                            single_packet=SINGLE_PACKET)
                        qctr[0] += 1
                        c0 += cn

            def load_idx(t, first=False):
                ncol = tile_cols[t]
                if first:
                    lo_i, hi_i, pr_i = lo_i0, hi_i0, pr_i0
                else:
                    lo_i = pool.tile([P, ncol], dt.int16)
                    nc.sync.dma_start(
                        out=lo_i[:],
                        in_=lo_idx_d[:, col_off[t]:col_off[t] + ncol])
                    hi_i = None
                    if has_hi[t]:
                        h = hi_num[t]
                        hi_i = pool.tile([P, 32 * s_hi], dt.int16)
                        nc.sync.dma_start(
                            out=hi_i[:],
                            in_=hi_idx_d[:, h * 32 * s_hi:(h + 1) * 32 * s_hi])
                    pr_i = None
                    if tile_pcols[t]:
                        pr_i = pool.tile([P, tile_pcols[t]], dt.int16)
                        nc.sync.dma_start(
                            out=pr_i[:],
                            in_=pr_idx_d[:, pcol_off[t]:pcol_off[t] + tile_pcols[t]])
                return lo_i, hi_i, pr_i

            # tile 0 gathers first (lead-in), then tail-tile gathers (prepaid)
            g_cache = {}
            lo_i, hi_i = load_idx(0, first=True)
            g_lo = gpool.tile([P, 1, planes_lo[0] * tiles[0]], dt.bfloat16,
                              name="g_lo")
            g_hi = (gpool.tile([P, 1, s_hi * 512], dt.bfloat16, name="g_hi")
                    if has_hi[0] else None)
            issue_gather(lo_i, hi_i, g_lo, g_hi, planes_lo[0] * tiles[0], has_hi[0])
            g_cache[0] = (g_lo, g_hi)

            for t in range(tail_from, n_tiles):
                nt = tiles[t]
                assert not has_hi[t]
                lo_i, _ = load_idx(t)
                g_lo = cpool.tile([P, 1, planes_lo[t] * nt], dt.bfloat16,
                                  name=f"g_tail{t}")
                issue_gather(lo_i, None, g_lo, None, planes_lo[t] * nt, False)
                g_cache[t] = (g_lo, None)

            for t, nt in enumerate(tiles):
                n0 = tile_off[t]
                pl = planes_lo[t]

                if t in g_cache:
                    g_lo, g_hi = g_cache.pop(t)
                else:
                    lo_i, hi_i = load_idx(t)
                    g_lo = gpool.tile([P, 1, pl * nt], dt.bfloat16,
                                      name="g_lo")
                    g_hi = (gpool.tile([P, 1, s_hi * 512], dt.bfloat16,
                                       name="g_hi")
                            if has_hi[t] else None)
                    issue_gather(lo_i, hi_i, g_lo, g_hi, pl * nt, has_hi[t])

                psum_a = ppool.tile([P, nt], dt.float32)
                n_planes = pl + (s_hi if has_hi[t] else 0)
                pi = 0
                for s in range(pl):
                    nc.tensor.matmul(
                        psum_a[:], lhsT=ident[:],
                        rhs=g_lo[:, 0, s * nt:(s + 1) * nt],
                        start=(pi == 0), stop=(pi == n_planes - 1))
                    pi += 1
                if has_hi[t]:
                    for s in range(s_hi):
                        nc.tensor.matmul(
                            psum_a[:], lhsT=ident[:],
                            rhs=g_hi[:, 0, s * 512:s * 512 + nt],
                            start=(pi == 0), stop=(pi == n_planes - 1))
                        pi += 1

                gsum = pool.tile([P, nt], dt.bfloat16)
                nc.vector.tensor_copy(out=gsum[:], in_=psum_a[:])

                x0_t = pool.tile([P, nt], dt.bfloat16)
                nc.sync.dma_start(out=x0_t[:], in_=x0t_d[:, n0:n0 + nt])
                xs_t = pool.tile([P, nt], dt.bfloat16)
                nc.sync.dma_start(out=xs_t[:], in_=xself_d[:, n0:n0 + nt])

                psum_b = ppool.tile([P, nt], dt.float32)
                nc.tensor.matmul(psum_b[:], lhsT=m1t[:], rhs=gsum[:],
                                 start=True, stop=False)
                nc.tensor.matmul(psum_b[:], lhsT=m1t[:], rhs=xs_t[:],
                                 start=False, stop=False)
                nc.tensor.matmul(psum_b[:], lhsT=m2t[:], rhs=x0_t[:],
                                 start=False, stop=True)

                out_t = pool.tile([P, nt], dt.bfloat16)
                nc.scalar.activation(
                    out_t[:], psum_b[:], mybir.ActivationFunctionType.Relu,
                    bias=biasv[:, 0:1], scale=1.0)
                nc.sync.dma_start(out=out_d[:, n0:n0 + nt], in_=out_t[:])
    nc.compile()
    return nc


# --------------------------------------------------------------------------
# full host prep (shared by kernel() and tests)
# --------------------------------------------------------------------------

def _prepare(x, x_0, edge_index, W1, W2, bias, n_cores, lo_cap, s_hi_try=(1, 2, 3, 4, 6, 8)):
    x = np.asarray(x, dtype=F32)          # [1, C, N, 1]
    x_0 = np.asarray(x_0, dtype=F32)      # [1, N, C]
    ei = np.asarray(edge_index)           # [2, 1, N, K]
    W1 = np.asarray(W1, dtype=F32)
    W2 = np.asarray(W2, dtype=F32)
    bias = np.asarray(bias, dtype=F32)

    n_rows = x.shape[2]
    nsh = n_rows // n_cores
    idx_all = np.asarray(ei[0, 0], dtype=np.int64)   # [N, K]
    K = idx_all.shape[1]
    assert K == DEG_K

    x_cn = np.ascontiguousarray(x[0, :, :, 0])       # [C, N]
    x_nm = np.ascontiguousarray(x_cn.T)              # [N, C]
    x_bf16 = x_nm.astype(BF16)
    x0_cn = np.ascontiguousarray(x_0[0].T)           # [C, N]

    deg = K + 1
    s1 = (1.0 - ALPHA) * (1.0 - BETA)
    s2 = ALPHA * (1.0 - BETA)
    eye = np.eye(C, dtype=np.float64)
    m1sT = ((s1 * eye + BETA * W1.astype(np.float64)).T / deg).astype(BF16)
    m2T = ((s2 * eye + BETA * W2.astype(np.float64)).T).astype(BF16)
    bias_v = np.ascontiguousarray(bias.reshape(-1)[:, None].astype(F32))
    ident = np.eye(P, dtype=BF16)

    tiles = _split_tiles(nsh)
    hi_needed = n_rows - (lo_cap - 1)
    hi_cap = 0
    if hi_needed > 0:
        hi_cap = ((hi_needed + 1 + P - 1) // P) * P

    infos = None
    s_hi_used = None
    for s_hi in s_hi_try:
        infos = []
        ok = True
        for c in range(n_cores):
            sl = slice(c * nsh, (c + 1) * nsh)
            info = _core_hi_info(x_bf16, idx_all[sl], nsh, n_rows, lo_cap, s_hi)
            if info is None:
                ok = False
                break
            infos.append(info)
        if ok:
            s_hi_used = s_hi
            break
    assert s_hi_used is not None, "could not find feasible s_hi"
    s_hi = s_hi_used

    # shared tile structure across cores (SPMD: one program for all).
    # hi-node block occupies node positions [head_lo, head_lo + hi_count_c)
    hi_counts = [info["n_hi_nodes"] for info in infos]
    head_lo = nsh % 512 if SMALL_FIRST else 0
    cum = np.cumsum([0] + tiles)
    planes_lo = []
    has_hi = []
    for t, nt in enumerate(tiles):
        pure = (cum[t] >= head_lo and cum[t + 1] <= head_lo + min(hi_counts)
                and nt == 512)
        hi = cum[t + 1] > head_lo and cum[t] < head_lo + max(hi_counts)
        planes_lo.append(DEG_K - 1 if pure else DEG_K)
        has_hi.append(bool(hi))
    # hi tiles must be full 512 tiles (hi grid planes are 512 wide)
    for t, h in enumerate(has_hi):
        if h:
            assert tiles[t] == 512, (tiles, has_hi)

    # pair planes per tile: limited by the weakest node in the tile across
    # all cores (pair gathers need num_idxs % 128 == 0 -> 512-node tiles)
    planes_pair = []
    for t, nt in enumerate(tiles):
        if nt != 512:
            planes_pair.append(0)
            continue
        pmin = min(int(info["p_cnt"][info["perm"]][cum[t]:cum[t + 1]].min())
                   for info in infos)
        planes_pair.append(min(pmin, planes_lo[t] // 2))

    in_maps = []
    perms = []
    for c in range(n_cores):
        sl = slice(c * nsh, (c + 1) * nsh)
        info = infos[c]
        perm = info["perm"]
        perms.append(perm)
        d = _pack_core(info, nsh, lo_cap, hi_cap, s_hi, tiles, planes_lo,
                       has_hi, planes_pair)
        n_ids = lo_cap + hi_cap
        table = np.zeros((n_ids, C), dtype=BF16)
        table[info["lid"]] = x_bf16
        gsl = np.arange(c * nsh, (c + 1) * nsh)[perm]  # global node ids, perm order
        in_maps.append(dict(
            table=table,
            tablep=np.ascontiguousarray(table.reshape(n_ids // 2, 2 * C)),
            lo_idx=d["lo_idx"],
            pr_idx=d["pr_idx"],
            hi_idx=d["hi_idx"],
            x0t=np.ascontiguousarray(x0_cn[:, gsl]).astype(BF16),
            xself=np.ascontiguousarray(x_cn[:, gsl]).astype(BF16),
            m1t=m1sT,
            m2t=m2T,
            biasv=bias_v,
            ident=ident,
        ))
    meta = dict(nsh=nsh, tiles=tiles, lo_cap=lo_cap, hi_cap=hi_cap,
                s_hi=s_hi, n_rows=n_rows, planes_lo=planes_lo, has_hi=has_hi,
                planes_pair=planes_pair, perms=perms)
    return in_maps, meta


last_results = None  # BassKernelResults of the most recent kernel() call


def kernel(x, x_0, edge_index, W1, W2, bias):
    global last_results
    import os
    in_maps, meta = _prepare(x, x_0, edge_index, W1, W2, bias,
                             n_cores=N_CORES, lo_cap=LO_CAP_FULL)
    nc = _build_program(meta["nsh"], meta["tiles"], meta["lo_cap"],
                        meta["hi_cap"], meta["s_hi"], meta["planes_lo"],
                        meta["has_hi"], meta["planes_pair"])
    trace = os.environ.get("GCNII_TRACE", "") == "1"
    res = run_bass_kernel_spmd(nc, in_maps, core_ids=list(range(N_CORES)),
                               trace=trace)
    last_results = res
    outs = []
    for c, r in enumerate(res.results):
        o = np.empty_like(r["out"])
        o[:, meta["perms"][c]] = r["out"]   # undo node permutation
        outs.append(o)
    out = np.concatenate(outs, axis=1)
    return np.ascontiguousarray(out.astype(F32))[None, :, :, None]


# revision 5
# speedup vs baseline: 1.0312x; 1.0010x over previous
"""GCNII conv (gnn_message_passing) Trainium2 Bass kernel.

Strategy (8-way node sharding, DRAM-resident relabeled feature table):
  - Host: relabel node-rows per core so gather indices fit int16 windows:
    "lo" window = table rows [0, 32768) (row 0 zeros), "hi" window = rows
    [32768, 40960) (row 32768 zeros).  Rows are assigned so each node has at
    most s_hi refs into the hi window.  Nodes with a hi ref are permuted to
    the FRONT of the core's shard so only the leading tiles need a hi gather
    plane; tiles made purely of hi-nodes drop their all-padding 16th lo
    plane.  The inverse permutation is applied to the output on host.
  - Device: dma_gather in transpose mode sources 256B rows directly from the
    DRAM table (no SBUF table copy); PE sums the neighbor slots plus self via
    bf16 identity matmuls into PSUM (fp32 exact), then the GCNII combine is
    bf16 GEMMs (M1s = (s1*I + beta*W1)/deg on gather_sum + x_self, M2 =
    s2*I + beta*W2 on x_0) plus bias+ReLU on the activation engine.
"""

import numpy as np
import ml_dtypes

import concourse.bacc as bacc
import concourse.mybir as mybir
from concourse.tile import TileContext
from concourse.bass_utils import run_bass_kernel_spmd

BF16 = ml_dtypes.bfloat16
F32 = np.float32

ALPHA = 0.1
BETA = float(np.log(0.5 / 4 + 1.0))
DEG_K = 16           # neighbors per node (w/o self loop)
C = 128              # channels
P = 128              # partitions

N_FULL = 40000
N_CORES = 8
LO_CAP_FULL = 32768  # rows in lo window (incl zero row at local id 0)

GATHER_CHUNK = 896   # idxs per dma_gather instruction (ucode cap ~992, %128)
SINGLE_PACKET = True
TAIL_PREFETCH = False  # issue tail-tile gathers early (helps only if DMA slack)
SPLIT_TAIL = True      # split the last tile to shorten the exposed end chain
SMALL_FIRST = False    # head bubbles block the critical path; keep small tiles last
RAMP_FIRST = False     # progressive tile-0 chunks regressed in sim
SHARED_REGS = True     # one num_idxs register per distinct chunk size


# --------------------------------------------------------------------------
# host-side preparation
# --------------------------------------------------------------------------

def _choose_hi_rows(refs, owners, n_rows, nsh, hi_needed, s_hi):
    """Pick `hi_needed` rows for the hi window s.t. no node has more than
    `s_hi` references into the hi window.  Prefers cold rows."""
    counts = np.bincount(refs, minlength=n_rows)
    order = np.argsort(counts, kind="stable")
    si = np.argsort(refs, kind="stable")
    owners_s = owners[si]
    starts = np.searchsorted(refs[si], np.arange(n_rows))
    ends = np.searchsorted(refs[si], np.arange(n_rows) + 1)

    is_hi = np.zeros(n_rows, dtype=bool)
    node_cnt = np.zeros(nsh, dtype=np.int64)
    zero_rows = order[counts[order] == 0]
    take = zero_rows[: min(len(zero_rows), hi_needed)]
    is_hi[take] = True
    n_hi = len(take)
    if n_hi < hi_needed:
        for r in order:
            if counts[r] == 0 or is_hi[r]:
                continue
            ow = owners_s[starts[r]:ends[r]]
            u, m = np.unique(ow, return_counts=True)
            if (node_cnt[u] + m <= s_hi).all():
                node_cnt[u] += m
                is_hi[r] = True
                n_hi += 1
                if n_hi == hi_needed:
                    break
    if n_hi != hi_needed:
        return None
    return is_hi


def _choose_hi_rows_masked(refs, owners, n_rows, nsh, hi_needed, s_hi, counts):
    """_choose_hi_rows with externally supplied counts (paired rows masked)."""
    order = np.argsort(counts, kind="stable")
    si = np.argsort(refs, kind="stable")
    owners_s = owners[si]
    starts = np.searchsorted(refs[si], np.arange(n_rows))
    ends = np.searchsorted(refs[si], np.arange(n_rows) + 1)

    is_hi = np.zeros(n_rows, dtype=bool)
    node_cnt = np.zeros(nsh, dtype=np.int64)
    zero_rows = order[counts[order] == 0]
    take = zero_rows[: min(len(zero_rows), hi_needed)]
    is_hi[take] = True
    n_hi = len(take)
    if n_hi < hi_needed:
        for r in order:
            if counts[r] == 0 or counts[r] >= (1 << 29) or is_hi[r]:
                continue
            ow = owners_s[starts[r]:ends[r]]
            u, m = np.unique(ow, return_counts=True)
            if (node_cnt[u] + m <= s_hi).all():
                node_cnt[u] += m
                is_hi[r] = True
                n_hi += 1
                if n_hi == hi_needed:
                    break
    if n_hi != hi_needed:
        return None
    return is_hi


def _split_tiles(nsh):
    """Tile sizes; small tail tile shortens the exposed end-of-kernel chain."""
    n_full, left = divmod(nsh, 512)
    if left == 0:
        small = []
    elif SPLIT_TAIL and left > 256 and left % 8 == 0 and (left - 256) % 8 == 0:
        # 256 is %128 -> the bigger tail tile stays pair-gather eligible
        small = [256, left - 256]
    elif SPLIT_TAIL and left > 128 and left % 8 == 0 and (left - 128) % 8 == 0 \
            and left - 128 >= 8:
        small = [128, left - 128] if SMALL_FIRST else [left - 128, 128]
    else:
        small = [left]
    tiles = small + [512] * n_full if SMALL_FIRST else [512] * n_full + small
    assert sum(tiles) == nsh
    return tiles


def _greedy_match(refs_n, n_rows, eligible):
    """Greedy matching on the co-reference graph restricted to eligible rows:
    pair rows that some node references together.  partner[r] = row or -1."""
    nsh, K = refs_n.shape
    us, vs = [], []
    for a in range(K):
        for b in range(a + 1, K):
            u = refs_n[:, a]
            v = refs_n[:, b]
            m = (u != v) & eligible[u] & eligible[v]
            uu = np.minimum(u[m], v[m])
            vv = np.maximum(u[m], v[m])
            us.append(uu)
            vs.append(vv)
    U = np.concatenate(us)
    V = np.concatenate(vs)
    partner = np.full(n_rows, -1, dtype=np.int64)
    rng = np.random.default_rng(0)
    for _ in range(12):
        free = (partner[U] < 0) & (partner[V] < 0)
        U, V = U[free], V[free]
        if len(U) == 0:
            break
        sh = rng.permutation(len(U))
        U, V = U[sh], V[sh]
        o = np.argsort(U, kind="stable")
        U, V = U[o], V[o]
        fu = np.ones(len(U), bool)
        fu[1:] = U[1:] != U[:-1]
        U1, V1 = U[fu], V[fu]
        o2 = np.argsort(V1, kind="stable")
        U1, V1 = U1[o2], V1[o2]
        fv = np.ones(len(V1), bool)
        fv[1:] = V1[1:] != V1[:-1]
        U2, V2 = U1[fv], V1[fv]
        ok = ~np.isin(U2, V2) & ~np.isin(V2, U2)
        partner[U2[ok]] = V2[ok]
        partner[V2[ok]] = U2[ok]
    return partner


def _core_hi_info(x_bf16, idx_shard, nsh, n_rows, lo_cap, s_hi):
    """Phase 1: hi rows, row pairing among lo rows, node permutation."""
    K = idx_shard.shape[1]
    refs_n = idx_shard.astype(np.int64)              # [nsh, K]
    refs = refs_n.reshape(-1)
    owners = np.repeat(np.arange(nsh, dtype=np.int64), K)
    # ids: [zero pair (2)] [paired rows] [unpaired lo rows] | [hi rows]
    hi_needed = max(0, n_rows + 2 - lo_cap)
    if hi_needed > 0:
        is_hi = _choose_hi_rows(refs, owners, n_rows, nsh, hi_needed, s_hi)
        if is_hi is None:
            return None
    else:
        is_hi = np.zeros(n_rows, dtype=bool)

    partner = _greedy_match(refs_n, n_rows, ~is_hi)

    # per-node pair selection over ref occurrences (each occurrence used once)
    used = np.zeros((nsh, K), dtype=bool)
    p_cnt = np.zeros(nsh, dtype=np.int64)
    for a in range(K):
        for b in range(a + 1, K):
            u = refs_n[:, a]
            v = refs_n[:, b]
            cand = (~used[:, a] & ~used[:, b] & (partner[u] == v)
                    & (u != v) & (p_cnt < K // 2))
            if not cand.any():
                continue
            used[cand, a] = True
            used[cand, b] = True
            p_cnt[cand] += 1

    # rows actually pair-fetched: keep ALL matched row pairs in the pair
    # region (a matched row might also be single-fetched; its id stays
    # < 2*(n_pairs+1) < lo_cap so the lo singles window reaches it)
    matched = partner >= 0
    plist = np.where(matched & (partner > np.arange(n_rows)))[0]
    n_pairs = len(plist)
    pair_id = np.full(n_rows, -1, dtype=np.int64)
    pair_id[plist] = 1 + np.arange(n_pairs)          # pair 0 = zero pad pair
    pair_id[partner[plist]] = pair_id[plist]

    lid = np.empty(n_rows, dtype=np.int64)
    lid[plist] = 2 * pair_id[plist]
    lid[partner[plist]] = 2 * pair_id[plist] + 1
    unp_lo = np.where(~matched & ~is_hi)[0]
    base = 2 * (n_pairs + 1)
    lid[unp_lo] = base + np.arange(len(unp_lo))
    hi_rows = np.where(is_hi)[0]
    lid[hi_rows] = lo_cap + 1 + np.arange(len(hi_rows))
    assert base + len(unp_lo) <= lo_cap, (base, len(unp_lo))
    assert n_pairs + 1 <= 16384, n_pairs

    lids_n = lid[refs_n]
    h_n = (lids_n >= lo_cap).sum(axis=1)
    # node order: hi-nodes first, then rest; within each group pair-count desc
    key = (h_n == 0).astype(np.int64) * (K + 2) + (K // 2 - p_cnt)
    perm = np.argsort(key, kind="stable")
    return dict(lid=lid, lids_n=lids_n, h_n=h_n, perm=perm,
                n_hi_nodes=int((h_n > 0).sum()), p_cnt=p_cnt,
                pair_id=pair_id, used=used, refs_n=refs_n,
                n_pairs=n_pairs)


def _pack_core(info, nsh, lo_cap, hi_cap, s_hi, tiles, planes_lo, has_hi,
               planes_pair):
    """Phase 2: build index grids given the shared tile structure."""
    lid = info["lid"]
    perm = info["perm"]
    refs_n = info["refs_n"][perm]        # [nsh, K] global rows, permuted
    used = info["used"][perm]
    h_n = info["h_n"][perm]
    p_cnt = info["p_cnt"][perm]
    pair_id = info["pair_id"]
    K = refs_n.shape[1]
    lids_n = lid[refs_n]

    lo_cols = sum(nt * (pl - 2 * pp) // 16
                  for nt, pl, pp in zip(tiles, planes_lo, planes_pair))
    pr_cols = sum(nt * pp // 16 for nt, pp in zip(tiles, planes_pair))
    n_hi_tiles = int(sum(has_hi))
    lo_idx = np.zeros((16, max(lo_cols, 16)), dtype=np.int16)
    pr_idx = np.zeros((16, max(pr_cols, 16)), dtype=np.int16)
    hi_idx = np.zeros((16, 32 * s_hi * max(n_hi_tiles, 1)), dtype=np.int16)
    off = 0
    lcol = 0
    pcol = 0
    hi_t = 0
    for t, nt in enumerate(tiles):
        pl = planes_lo[t]
        pp = planes_pair[t]
        sl_r = refs_n[off:off + nt]
        sl_u = used[off:off + nt].copy()
        sl_h = h_n[off:off + nt]
        assert (p_cnt[off:off + nt] >= pp).all(), (t, pp)
        # pair slots: first pp pairs of each node -> pair ids
        pvals = np.zeros((nt, pp), dtype=np.int64)
        consumed = np.zeros_like(sl_u)
        if pp:
            taken = np.zeros(nt, dtype=np.int64)
            for a in range(K):
                for b in range(a + 1, K):
                    u = sl_r[:, a]
                    v = sl_r[:, b]
                    cand = (sl_u[:, a] & sl_u[:, b]
                            & ~consumed[:, a] & ~consumed[:, b]
                            & (pair_id[u] >= 0)
                            & (pair_id[u] == pair_id[v]) & (u != v)
                            & (taken < pp))
                    if not cand.any():
                        continue
                    pvals[cand, taken[cand]] = pair_id[u[cand]]
                    consumed[cand, a] = True
                    consumed[cand, b] = True
                    taken[cand] += 1
            assert (taken == pp).all(), (t, pp, taken.min())
            flatp = pvals.T.reshape(-1)
            ncolp = pp * nt // 16
            pr_idx[:, pcol:pcol + ncolp] = flatp.reshape(-1, 16).T
            pcol += ncolp
        # singles: all lo-ref occurrences not consumed as pairs (includes
        # never-paired occurrences and leftover unused pairs)
        n_sing = pl - 2 * pp
        svals = np.zeros((nt, n_sing), dtype=np.int64)
        islo = lids_n[off:off + nt] < lo_cap
        for j_col in range(nt):
            occ = np.where(~consumed[j_col] & islo[j_col])[0]
            vals = lids_n[off + j_col, occ]
            assert len(vals) <= n_sing, (t, j_col, len(vals), n_sing)
            svals[j_col, :len(vals)] = vals
        flat = svals.T.reshape(-1)
        ncol = n_sing * nt // 16
        lo_idx[:, lcol:lcol + ncol] = flat.reshape(-1, 16).T
        lcol += ncol
        # hi plane
        if has_hi[t]:
            assert nt == 512
            hvals = np.zeros((s_hi, nt), dtype=np.int64)
            ishi_n = lids_n[off:off + nt] >= lo_cap
            for j_col in range(nt):
                occ = np.where(ishi_n[j_col])[0]
                assert len(occ) <= s_hi
                for s, o in enumerate(occ):
                    hvals[s, j_col] = lids_n[off + j_col, o] - lo_cap
            hflat = np.zeros(s_hi * 512, dtype=np.int64)
            for s in range(s_hi):
                hflat[s * 512: s * 512 + nt] = hvals[s]
            hi_idx[:, hi_t * 32 * s_hi:(hi_t + 1) * 32 * s_hi] = (
                hflat.reshape(-1, 16).T)
            hi_t += 1
        else:
            assert (sl_h == 0).all(), t
        off += nt
    assert off == nsh
    return dict(lo_idx=np.tile(lo_idx, (8, 1)),
                pr_idx=np.tile(pr_idx, (8, 1)),
                hi_idx=np.tile(hi_idx, (8, 1)))


# --------------------------------------------------------------------------
# device program
# --------------------------------------------------------------------------

def _build_program(nsh, tiles, lo_cap, hi_cap, s_hi, planes_lo, has_hi,
                   planes_pair):
    dt = mybir.dt
    nc = bacc.Bacc("TRN2", target_bir_lowering=False, num_swdge_queues=N_QUEUES)
    n_ids = lo_cap + hi_cap
    K = DEG_K

    lo_cols = sum(nt * (pl - 2 * pp) // 16
                  for nt, pl, pp in zip(tiles, planes_lo, planes_pair))
    pr_cols = sum(nt * pp // 16 for nt, pp in zip(tiles, planes_pair))
    n_hi_tiles = int(sum(has_hi))

    table_d = nc.dram_tensor("table", [n_ids, C], dt.bfloat16, kind="ExternalInput")
    tablep_d = nc.dram_tensor("tablep", [n_ids // 2, 2 * C], dt.bfloat16,
                              kind="ExternalInput")
    lo_idx_d = nc.dram_tensor("lo_idx", [P, max(lo_cols, 16)], dt.int16,
                              kind="ExternalInput")
    pr_idx_d = nc.dram_tensor("pr_idx", [P, max(pr_cols, 16)], dt.int16,
                              kind="ExternalInput")
    hi_idx_d = nc.dram_tensor("hi_idx", [P, 32 * s_hi * max(n_hi_tiles, 1)],
                              dt.int16, kind="ExternalInput")
    x0t_d = nc.dram_tensor("x0t", [P, nsh], dt.bfloat16, kind="ExternalInput")
    xself_d = nc.dram_tensor("xself", [P, nsh], dt.bfloat16, kind="ExternalInput")
    m1t_d = nc.dram_tensor("m1t", [P, C], dt.bfloat16, kind="ExternalInput")
    m2t_d = nc.dram_tensor("m2t", [P, C], dt.bfloat16, kind="ExternalInput")
    bias_d = nc.dram_tensor("biasv", [P, 1], dt.float32, kind="ExternalInput")
    ident_d = nc.dram_tensor("ident", [P, P], dt.bfloat16, kind="ExternalInput")
    out_d = nc.dram_tensor("out", [P, nsh], dt.bfloat16, kind="ExternalOutput")

    with TileContext(nc) as tc:
        with (
            tc.tile_pool(name="consts", bufs=1) as cpool,
            tc.tile_pool(name="work", bufs=WORK_BUFS) as pool,
            tc.tile_pool(name="gpool", bufs=G_BUFS) as gpool,
            tc.tile_pool(name="psum", bufs=PSUM_BUFS, space="PSUM") as ppool,
        ):
            # issue tile-0 index loads before the consts so the first gather
            # starts as early as possible
            ncol0 = tiles[0] * (planes_lo[0] - 2 * planes_pair[0]) // 16
            pcol0 = tiles[0] * planes_pair[0] // 16
            pr_i0 = None
            if pcol0:
                pr_i0 = pool.tile([P, pcol0], dt.int16)
                nc.sync.dma_start(out=pr_i0[:], in_=pr_idx_d[:, 0:pcol0])
            lo_i0 = pool.tile([P, ncol0], dt.int16)
            nc.sync.dma_start(out=lo_i0[:], in_=lo_idx_d[:, 0:ncol0])
            hi_i0 = None
            if has_hi[0]:
                hi_i0 = pool.tile([P, 32 * s_hi], dt.int16)
                nc.sync.dma_start(out=hi_i0[:], in_=hi_idx_d[:, 0:32 * s_hi])

            m1t = cpool.tile([P, C], dt.bfloat16)
            nc.sync.dma_start(out=m1t[:], in_=m1t_d[:])
            m2t = cpool.tile([P, C], dt.bfloat16)
            nc.sync.dma_start(out=m2t[:], in_=m2t_d[:])
            biasv = cpool.tile([P, 1], dt.float32)
            nc.sync.dma_start(out=biasv[:], in_=bias_d[:])
            ident = cpool.tile([P, P], dt.bfloat16)
            nc.sync.dma_start(out=ident[:], in_=ident_d[:])

            n_tiles = len(tiles)
            # tail tiles: gathers prefetched early into persistent buffers so
            # only their (small) compute chain is exposed at kernel end
            tail_from = n_tiles - 2 if (TAIL_PREFETCH and n_tiles >= 3) else n_tiles
            tile_cols = [nt * (planes_lo[t] - 2 * planes_pair[t]) // 16
                         for t, nt in enumerate(tiles)]
            tile_pcols = [nt * planes_pair[t] // 16 for t, nt in enumerate(tiles)]
            tile_off = np.cumsum([0] + list(tiles))
            col_off = np.cumsum([0] + tile_cols)
            pcol_off = np.cumsum([0] + tile_pcols)
            hi_num = np.cumsum([0] + [int(h) for h in has_hi])

            def issue_gather(lo_i, hi_i, pr_i, g_lo, g_hi, g_pr, n_lo, n_pp,
                             nt, use_hi, ramp=False):
                if n_pp:
                    npr = n_pp * nt
                    nc.gpsimd.dma_gather(
                        out_ap=g_pr[:],
                        in_ap=tablep_d[0:16384, :],
                        idxs_ap=pr_i[:, 0:npr // 16],
                        num_idxs=npr, num_idxs_reg=idx_reg(npr),
                        elem_size=2 * C, transpose=True,
                        queue_num=qctr[0] % N_QUEUES,
                        single_packet=SINGLE_PACKET and npr <= GATHER_CHUNK)
                    qctr[0] += 1
                CH = GATHER_CHUNK
                ramp_sizes = [128, 256, 512] if ramp and RAMP_FIRST else []
                c0 = 0
                while c0 < n_lo:
                    cn = min(ramp_sizes.pop(0) if ramp_sizes else CH, n_lo - c0)
                    nc.gpsimd.dma_gather(
                        out_ap=g_lo[:, :, c0:c0 + cn],
                        in_ap=table_d[0:lo_cap, :],
                        idxs_ap=lo_i[:, c0 // 16:(c0 + cn) // 16],
                        num_idxs=cn, num_idxs_reg=idx_reg(cn), elem_size=C,
                        transpose=True, queue_num=qctr[0] % N_QUEUES,
                        single_packet=SINGLE_PACKET)
                    qctr[0] += 1
                    c0 += cn
                if use_hi:
                    n_hi = s_hi * 512
                    c0 = 0
                    while c0 < n_hi:
                        cn = min(CH, n_hi - c0)
                        nc.gpsimd.dma_gather(
                            out_ap=g_hi[:, :, c0:c0 + cn],
                            in_ap=table_d[lo_cap:, :],
                            idxs_ap=hi_i[:, c0 // 16:(c0 + cn) // 16],
                            num_idxs=cn, num_idxs_reg=idx_reg(cn), elem_size=C,
                            transpose=True, queue_num=qctr[0] % N_QUEUES,
                            single_packet=SINGLE_PACKET)
                        qctr[0] += 1
                        c0 += cn

            def load_idx(t, first=False):
                ncol = tile_cols[t]
                if first:
                    lo_i, hi_i, pr_i = lo_i0, hi_i0, pr_i0
                else:
                    lo_i = pool.tile([P, ncol], dt.int16)
                    nc.sync.dma_start(
                        out=lo_i[:],
                        in_=lo_idx_d[:, col_off[t]:col_off[t] + ncol])
                    hi_i = None
                    if has_hi[t]:
                        h = hi_num[t]
                        hi_i = pool.tile([P, 32 * s_hi], dt.int16)
                        nc.sync.dma_start(
                            out=hi_i[:],
                            in_=hi_idx_d[:, h * 32 * s_hi:(h + 1) * 32 * s_hi])
                    pr_i = None
                    if tile_pcols[t]:
                        pr_i = pool.tile([P, tile_pcols[t]], dt.int16)
                        nc.sync.dma_start(
                            out=pr_i[:],
                            in_=pr_idx_d[:, pcol_off[t]:pcol_off[t] + tile_pcols[t]])
                return lo_i, hi_i, pr_i

            # tile 0 gathers first (lead-in), then tail-tile gathers (prepaid)
            g_cache = {}
            lo_i, hi_i = load_idx(0, first=True)
            g_lo = gpool.tile([P, 1, planes_lo[0] * tiles[0]], dt.bfloat16,
                              name="g_lo")
            g_hi = (gpool.tile([P, 1, s_hi * 512], dt.bfloat16, name="g_hi")
                    if has_hi[0] else None)
            issue_gather(lo_i, hi_i, g_lo, g_hi, planes_lo[0] * tiles[0], has_hi[0])
            g_cache[0] = (g_lo, g_hi)

            for t in range(tail_from, n_tiles):
                nt = tiles[t]
                assert not has_hi[t]
                lo_i, _ = load_idx(t)
                g_lo = cpool.tile([P, 1, planes_lo[t] * nt], dt.bfloat16,
                                  name=f"g_tail{t}")
                issue_gather(lo_i, None, g_lo, None, planes_lo[t] * nt, False)
                g_cache[t] = (g_lo, None)

            for t, nt in enumerate(tiles):
                n0 = tile_off[t]
                pl = planes_lo[t]

                if t in g_cache:
                    g_lo, g_hi = g_cache.pop(t)
                else:
                    lo_i, hi_i = load_idx(t)
                    g_lo = gpool.tile([P, 1, pl * nt], dt.bfloat16,
                                      name="g_lo")
                    g_hi = (gpool.tile([P, 1, s_hi * 512], dt.bfloat16,
                                       name="g_hi")
                            if has_hi[t] else None)
                    issue_gather(lo_i, hi_i, g_lo, g_hi, pl * nt, has_hi[t])

                psum_a = ppool.tile([P, nt], dt.float32)
                n_planes = pl + (s_hi if has_hi[t] else 0)
                pi = 0
                for s in range(pl):
                    nc.tensor.matmul(
                        psum_a[:], lhsT=ident[:],
                        rhs=g_lo[:, 0, s * nt:(s + 1) * nt],
                        start=(pi == 0), stop=(pi == n_planes - 1))
                    pi += 1
                if has_hi[t]:
                    for s in range(s_hi):
                        nc.tensor.matmul(
                            psum_a[:], lhsT=ident[:],
                            rhs=g_hi[:, 0, s * 512:s * 512 + nt],
                            start=(pi == 0), stop=(pi == n_planes - 1))
                        pi += 1

                gsum = pool.tile([P, nt], dt.bfloat16)
                nc.vector.tensor_copy(out=gsum[:], in_=psum_a[:])

                x0_t = pool.tile([P, nt], dt.bfloat16)
                nc.sync.dma_start(out=x0_t[:], in_=x0t_d[:, n0:n0 + nt])
                xs_t = pool.tile([P, nt], dt.bfloat16)
                nc.sync.dma_start(out=xs_t[:], in_=xself_d[:, n0:n0 + nt])

                psum_b = ppool.tile([P, nt], dt.float32)
                nc.tensor.matmul(psum_b[:], lhsT=m1t[:], rhs=gsum[:],
                                 start=True, stop=False)
                nc.tensor.matmul(psum_b[:], lhsT=m1t[:], rhs=xs_t[:],
                                 start=False, stop=False)
                nc.tensor.matmul(psum_b[:], lhsT=m2t[:], rhs=x0_t[:],
                                 start=False, stop=True)

                out_t = pool.tile([P, nt], dt.bfloat16)
                nc.scalar.activation(
                    out_t[:], psum_b[:], mybir.ActivationFunctionType.Relu,
                    bias=biasv[:, 0:1], scale=1.0)
                nc.sync.dma_start(out=out_d[:, n0:n0 + nt], in_=out_t[:])
    nc.compile()
    return nc


# --------------------------------------------------------------------------
# full host prep (shared by kernel() and tests)
# --------------------------------------------------------------------------

def _prepare(x, x_0, edge_index, W1, W2, bias, n_cores, lo_cap, s_hi_try=(1, 2, 3, 4, 6, 8)):
    x = np.asarray(x, dtype=F32)          # [1, C, N, 1]
    x_0 = np.asarray(x_0, dtype=F32)      # [1, N, C]
    ei = np.asarray(edge_index)           # [2, 1, N, K]
    W1 = np.asarray(W1, dtype=F32)
    W2 = np.asarray(W2, dtype=F32)
    bias = np.asarray(bias, dtype=F32)

    n_rows = x.shape[2]
    nsh = n_rows // n_cores
    idx_all = np.asarray(ei[0, 0], dtype=np.int64)   # [N, K]
    K = idx_all.shape[1]
    assert K == DEG_K

    x_cn = np.ascontiguousarray(x[0, :, :, 0])       # [C, N]
    x_nm = np.ascontiguousarray(x_cn.T)              # [N, C]
    x_bf16 = x_nm.astype(BF16)
    x0_cn = np.ascontiguousarray(x_0[0].T)           # [C, N]

    deg = K + 1
    s1 = (1.0 - ALPHA) * (1.0 - BETA)
    s2 = ALPHA * (1.0 - BETA)
    eye = np.eye(C, dtype=np.float64)
    m1sT = ((s1 * eye + BETA * W1.astype(np.float64)).T / deg).astype(BF16)
    m2T = ((s2 * eye + BETA * W2.astype(np.float64)).T).astype(BF16)
    bias_v = np.ascontiguousarray(bias.reshape(-1)[:, None].astype(F32))
    ident = np.eye(P, dtype=BF16)

    tiles = _split_tiles(nsh)
    hi_needed = n_rows - (lo_cap - 1)
    hi_cap = 0
    if hi_needed > 0:
        hi_cap = ((hi_needed + 1 + P - 1) // P) * P

    infos = None
    s_hi_used = None
    for s_hi in s_hi_try:
        infos = []
        ok = True
        for c in range(n_cores):
            sl = slice(c * nsh, (c + 1) * nsh)
            info = _core_hi_info(x_bf16, idx_all[sl], nsh, n_rows, lo_cap, s_hi)
            if info is None:
                ok = False
                break
            infos.append(info)
        if ok:
            s_hi_used = s_hi
            break
    assert s_hi_used is not None, "could not find feasible s_hi"
    s_hi = s_hi_used

    # shared tile structure across cores (SPMD: one program for all).
    # hi-node block occupies node positions [head_lo, head_lo + hi_count_c)
    hi_counts = [info["n_hi_nodes"] for info in infos]
    head_lo = nsh % 512 if SMALL_FIRST else 0
    cum = np.cumsum([0] + tiles)
    planes_lo = []
    has_hi = []
    for t, nt in enumerate(tiles):
        pure = (cum[t] >= head_lo and cum[t + 1] <= head_lo + min(hi_counts)
                and nt == 512)
        hi = cum[t + 1] > head_lo and cum[t] < head_lo + max(hi_counts)
        planes_lo.append(DEG_K - 1 if pure else DEG_K)
        has_hi.append(bool(hi))
    # hi tiles must be full 512 tiles (hi grid planes are 512 wide)
    for t, h in enumerate(has_hi):
        if h:
            assert tiles[t] == 512, (tiles, has_hi)

    # pair planes per tile: limited by the weakest node in the tile across
    # all cores (pair gathers need num_idxs % 128 == 0 -> 512-node tiles)
    planes_pair = []
    for t, nt in enumerate(tiles):
        if nt % 128 != 0:
            planes_pair.append(0)
            continue
        pmin = min(int(info["p_cnt"][info["perm"]][cum[t]:cum[t + 1]].min())
                   for info in infos)
        planes_pair.append(min(pmin, planes_lo[t] // 2))

    in_maps = []
    perms = []
    for c in range(n_cores):
        sl = slice(c * nsh, (c + 1) * nsh)
        info = infos[c]
        perm = info["perm"]
        perms.append(perm)
        d = _pack_core(info, nsh, lo_cap, hi_cap, s_hi, tiles, planes_lo,
                       has_hi, planes_pair)
        n_ids = lo_cap + hi_cap
        table = np.zeros((n_ids, C), dtype=BF16)
        table[info["lid"]] = x_bf16
        gsl = np.arange(c * nsh, (c + 1) * nsh)[perm]  # global node ids, perm order
        in_maps.append(dict(
            table=table,
            tablep=np.ascontiguousarray(table.reshape(n_ids // 2, 2 * C)),
            lo_idx=d["lo_idx"],
            pr_idx=d["pr_idx"],
            hi_idx=d["hi_idx"],
            x0t=np.ascontiguousarray(x0_cn[:, gsl]).astype(BF16),
            xself=np.ascontiguousarray(x_cn[:, gsl]).astype(BF16),
            m1t=m1sT,
            m2t=m2T,
            biasv=bias_v,
            ident=ident,
        ))
    meta = dict(nsh=nsh, tiles=tiles, lo_cap=lo_cap, hi_cap=hi_cap,
                s_hi=s_hi, n_rows=n_rows, planes_lo=planes_lo, has_hi=has_hi,
                planes_pair=planes_pair, perms=perms)
    return in_maps, meta


last_results = None  # BassKernelResults of the most recent kernel() call


def kernel(x, x_0, edge_index, W1, W2, bias):
    global last_results
    import os
    in_maps, meta = _prepare(x, x_0, edge_index, W1, W2, bias,
                             n_cores=N_CORES, lo_cap=LO_CAP_FULL)
    nc = _build_program(meta["nsh"], meta["tiles"], meta["lo_cap"],
                        meta["hi_cap"], meta["s_hi"], meta["planes_lo"],
                        meta["has_hi"], meta["planes_pair"])
    trace = os.environ.get("GCNII_TRACE", "") == "1"
    res = run_bass_kernel_spmd(nc, in_maps, core_ids=list(range(N_CORES)),
                               trace=trace)
    last_results = res
    outs = []
    for c, r in enumerate(res.results):
        o = np.empty_like(r["out"])
        o[:, meta["perms"][c]] = r["out"]   # undo node permutation
        outs.append(o)
    out = np.concatenate(outs, axis=1)
    return np.ascontiguousarray(out.astype(F32))[None, :, :, None]
